# revision 30
# baseline (speedup 1.0000x reference)
"""Trainium2 Bass kernel for a single-head linear-projection attention block.

Reference computation (B=4, CH=256, N=4096):
    theta = Wt @ x        [B, 32, N]
    phi   = Wp @ x        [B, 32, N]
    g     = Wg @ x        [B, 128, N]
    scores = theta^T phi  [B, N, N]
    beta = softmax(scores, axis=-1)
    attn = g @ beta^T     [B, 128, N]
    out = gamma * (Wo @ attn) + x

Sharding: 8 cores = 4 batches x 2 query-halves. Each core owns one batch's
full sequence (for keys/values) and half the queries. The per-core x is
rotated so its query half is always columns 0:2048, keeping the SPMD program
identical across cores (softmax/attention are invariant to a consistent
permutation of the key axis). No collectives are needed.

Per-core dataflow (all matmuls bf16 with fp32 PSUM accumulation):
  - theta/phi via weight-stationary matmuls, column-packed so both run
    concurrently in distinct PE column groups; each is then replicated to a
    second 32-partition group (SBUF->SBUF DMA) for row-group score packing.
  - gT[m, c] via x-stationary matmuls (g transposed, m on partitions), since
    the attention matmul contracts over m which must sit on partitions; this
    dense back-to-back matmul burst also warms the PE clock (HAM) right
    before the main loop.
  - scoresT[m, n] = phi^T theta computed transposed so that softmax's exp
    output directly feeds the attention matmul without a transpose. Each
    PSUM slot holds a PAIR of m-tiles (one per bank); the two K=32 score
    matmuls run concurrently in distinct PE row groups.
  - exp on the Scalar engine, PSUM -> SBUF bf16 in [128, 1024] instructions
    (no max subtraction needed: |scores| <~ 30). The Scalar engine is the
    critical path; the main loop runs at its throughput (~1.0us/tile).
  - 4 passes of 512 queries; attention accumulated over m in PSUM; softmax
    denominator summed on the Vector engine (bf16 quad subtrees + fp32
    chain), partition-reduced+broadcast with one ones-matmul; normalization
    applied after the Wo projection so the reciprocal overlaps it. Each
    pass's epilogue is dripped into the next pass's loop to keep the PE
    queue gap-free (a >~0.7us PE gap permanently halves the PE clock).
  - gamma folded into Wo on the host; fp32 residual add with x.

Measured on 8 axon-tunneled TRN2 cores: ~104-107us, rel_err 5.5e-3.
"""

import os
import sys

import numpy as np

B, CH, N = 4, 256, 4096
NCORES = 8
NH = N // 2  # queries per core
P = 128

_REPO_CANDIDATES = ["/opt/trn_rl_repo", "/root/.axon_site/_ro/trn_rl_repo"]


def _ensure_import_path():
    try:
        import concourse.bass  # noqa: F401
        return
    except ImportError:
        pass
    for cand in _REPO_CANDIDATES:
        if os.path.isdir(cand):
            sys.path.insert(0, cand)
            try:
                import concourse.bass  # noqa: F401
                return
            except ImportError:
                sys.path.pop(0)
    raise ImportError("could not locate concourse (bass) repo")


_CACHE = {}


def build_bass():
    """Build + compile the per-core Tile program (identical on all 8 cores)."""
    _ensure_import_path()
    import concourse.bacc as bacc
    import concourse.tile as tile
    from concourse import mybir

    dt = mybir.dt
    f32 = dt.float32
    bf16 = dt.bfloat16
    Exp = mybir.ActivationFunctionType.Exp

    nc = bacc.Bacc(
        "TRN2",
        target_bir_lowering=False,
        debug=False,
        num_devices=NCORES,
    )

    # Per-core DRAM I/O.
    x_d = nc.dram_tensor("x", [CH, N], bf16, kind="ExternalInput")
    xq_d = nc.dram_tensor("xq", [CH, NH], f32, kind="ExternalInput")
    wt_d = nc.dram_tensor("wt", [CH, 32], bf16, kind="ExternalInput")   # Wt^T
    wp_d = nc.dram_tensor("wp", [CH, 32], bf16, kind="ExternalInput")   # Wp^T
    wg_d = nc.dram_tensor("wg", [CH, 128], bf16, kind="ExternalInput")  # Wg^T
    wo_d = nc.dram_tensor("wo", [128, CH], bf16, kind="ExternalInput")  # (gamma*Wo)^T
    out_d = nc.dram_tensor("out", [CH, NH], f32, kind="ExternalOutput")

    MT = N // P  # 32 m-tiles
    NQ = 512     # query chunk per pass (4 passes over n)

    with tile.TileContext(nc) as tc:
        with (
            tc.tile_pool(name="const", bufs=1) as const,
            tc.tile_pool(name="xp", bufs=1) as xp,
            tc.tile_pool(name="proj", bufs=1) as proj,
            tc.tile_pool(name="expp", bufs=6) as expp,
            tc.tile_pool(name="acc", bufs=2) as acc,
            tc.tile_pool(name="outp", bufs=1) as outp,
            tc.tile_pool(name="tree", bufs=2) as tree,
            tc.tile_pool(name="ps2", bufs=3, space="PSUM") as ps2,
            tc.tile_pool(name="psA", bufs=2, space="PSUM") as psA,
        ):
            # ---- weights (tiny, scalar-engine HWDGE queue, needed first) ----
            wt_sb = const.tile([P, 2, 32], bf16)
            wp_sb = const.tile([P, 2, 32], bf16)
            wg_sb = const.tile([P, 2, 128], bf16)
            wo_sb = const.tile([P, CH], bf16)
            ones_sb = const.tile([P, P], bf16)
            nc.scalar.dma_start(
                out=wt_sb, in_=wt_d.ap().rearrange("(kb p) m -> p kb m", p=P)
            )
            nc.scalar.dma_start(
                out=wp_sb, in_=wp_d.ap().rearrange("(kb p) m -> p kb m", p=P)
            )
            nc.vector.memset(ones_sb, 1.0)

            # dense dummy matmul burst during the initial x-DMA wait: trips
            # the PE clock monitor (HAM) to full rate before the projections
            warm_sb = const.tile([P, 512], bf16)
            nc.vector.memset(warm_sb, 0.0)
            for _ in range(12):
                ps_w = ps2.tile([P, 512], f32, tag="ps")
                nc.tensor.matmul(ps_w, lhsT=ones_sb, rhs=warm_sb, start=True, stop=True)

            # ---- x in column blocks of 1024 so compute starts early ----
            x_sb = xp.tile([P, 2, N], bf16)
            xq_sb = xp.tile([P, 2, NH], f32)
            for cb in range(4):
                eng = nc.sync if cb < 2 else nc.scalar
                for kb in range(2):
                    eng.dma_start(
                        out=x_sb[:, kb, cb * 1024:(cb + 1) * 1024],
                        in_=x_d[kb * P:(kb + 1) * P, cb * 1024:(cb + 1) * 1024],
                    )
            # wg/wo are needed later (gT / epilogue): queue them behind x
            nc.sync.dma_start(
                out=wg_sb, in_=wg_d.ap().rearrange("(kb p) m -> p kb m", p=P)
            )
            nc.sync.dma_start(out=wo_sb, in_=wo_d.ap())

            # ---- projections, theta/phi column-packed (concurrent in PE col
            # groups): theta natively at partitions 0:32, phi at 32:64; the
            # replica DMAs fill the opposite group so both live at 0:64 for
            # the 2x row-group packing of the score matmuls. ----
            theta_sb = proj.tile([64, NH], bf16)
            phi_sb = proj.tile([64, N], bf16)
            gT_sb = proj.tile([P, MT, P], bf16)
            for cb in range(4):
                cbs = slice(cb * 1024, (cb + 1) * 1024)
                ps_p = ps2.tile([64, 1024], f32, tag="ps")
                for c in range(2):
                    for kb in range(2):
                        if cb < 2:
                            nc.tensor.matmul(
                                ps_p[0:32, c * 512:(c + 1) * 512],
                                lhsT=wt_sb[:, kb, :],
                                rhs=x_sb[:, kb, cb * 1024 + c * 512:cb * 1024 + (c + 1) * 512],
                                start=(kb == 0),
                                stop=(kb == 1),
                                skip_group_check=True,
                            )
                        nc.tensor.matmul(
                            ps_p[32:64, c * 512:(c + 1) * 512],
                            lhsT=wp_sb[:, kb, :],
                            rhs=x_sb[:, kb, cb * 1024 + c * 512:cb * 1024 + (c + 1) * 512],
                            start=(kb == 0),
                            stop=(kb == 1),
                            skip_group_check=True,
                        )
                if cb < 2:
                    nc.vector.tensor_copy(out=theta_sb[0:32, cbs], in_=ps_p[0:32, :])
                    nc.scalar.dma_start(
                        out=theta_sb[32:64, cbs], in_=theta_sb[0:32, cbs]
                    )
                nc.vector.tensor_copy(out=phi_sb[32:64, cbs], in_=ps_p[32:64, :])
                nc.scalar.dma_start(out=phi_sb[0:32, cbs], in_=phi_sb[32:64, cbs])

            # ---- gT last: a dense back-to-back matmul burst that warms the
            # PE clock (HAM) right before the gap-free main loop ----
            def emit_gt_group(grp):
                ps_g = ps2.tile([P, 4, P], f32, tag="ps")
                for j in range(4):
                    mt = grp * 4 + j
                    for kb in range(2):
                        nc.tensor.matmul(
                            ps_g[:, j, :],
                            lhsT=x_sb[:, kb, mt * P:(mt + 1) * P],
                            rhs=wg_sb[:, kb, :],
                            start=(kb == 0),
                            stop=(kb == 1),
                        )
                nc.vector.tensor_copy(
                    out=gT_sb[:, grp * 4:(grp + 1) * 4, :], in_=ps_g
                )

            for grp in range(MT // 4 - 1):
                emit_gt_group(grp)

            # residual x slice: only needed by the epilogue, so load it late
            for kb in range(2):
                nc.scalar.dma_start(
                    out=xq_sb[:, kb, :], in_=xq_d[kb * P:(kb + 1) * P, :]
                )

            out_sb = outp.tile([P, 2, NH], f32)

            def epilogue_pieces(nh, attn_ps, S_bf):
                """Piece 0 (the cast) frees the attention PSUM slot for the
                next pass immediately; the rest is dripped into the next
                pass's loop. Epilogue PSUM lives in its own 1-bank pool."""
                A_bf = acc.tile([P, NQ], bf16, tag="abf")
                nc.vector.tensor_copy(out=A_bf, in_=attn_ps)
                yield
                ps_S = psA.tile([P, NQ], f32, tag="attn")
                nc.tensor.matmul(ps_S, lhsT=ones_sb, rhs=S_bf, start=True, stop=True)
                yield
                recip = acc.tile([P, NQ], f32, tag="recip")
                nc.vector.reciprocal_approx_fast(out=recip, in_=ps_S)
                yield
                sl = slice(nh * NQ, (nh + 1) * NQ)
                for oc in range(2):
                    ps_o = psA.tile([P, NQ], f32, tag="attn")
                    nc.tensor.matmul(
                        ps_o,
                        lhsT=wo_sb[:, oc * P:(oc + 1) * P],
                        rhs=A_bf,
                        start=True,
                        stop=True,
                    )
                    tmp = acc.tile([P, NQ], f32, tag="tmp")
                    nc.vector.tensor_mul(tmp, ps_o, recip)
                    nc.vector.tensor_add(out_sb[:, oc, sl], tmp, xq_sb[:, oc, sl])
                    nc.sync.dma_start(
                        out=out_d[oc * P:(oc + 1) * P, sl], in_=out_sb[:, oc, sl]
                    )
                    yield

            # ---- four passes over the query axis (512 queries each). Each
            # PSUM slot holds one PAIR of m-tiles (mtE in cols 0:512 = bank 0,
            # mtO in cols 512:1024 = bank 1): the two row-group score matmuls
            # become ready together and run concurrently in the PE array.
            # Each pass's epilogue interleaves into the next pass's loop. ----
            pending = None
            for nh in range(4):
                attn_ps = psA.tile([P, NQ], f32, tag="attn")
                quads = {}
                chain = None
                for mtp in range(MT // 2):
                    mtE, mtO = mtp * 2, mtp * 2 + 1
                    ns = slice(nh * NQ, (nh + 1) * NQ)
                    ps_s = ps2.tile([P, 1024], f32, tag="ps")
                    for j, mt in ((0, mtE), (1, mtO)):
                        nc.tensor.matmul(
                            ps_s[:, j * 512:(j + 1) * 512],
                            lhsT=phi_sb[32 * j:32 * (j + 1), mt * P:(mt + 1) * P],
                            rhs=theta_sb[32 * j:32 * (j + 1), ns],
                            start=True,
                            stop=True,
                        )
                    if nh == 0 and mtp == 0:
                        # dense PE bridge over the first exp's latency
                        emit_gt_group(MT // 4 - 1)
                    expt = expp.tile([P, 1024], bf16, tag="expt")
                    nc.scalar.activation(out=expt, in_=ps_s, func=Exp)
                    for j, mt in ((0, mtE), (1, mtO)):
                        nc.tensor.matmul(
                            attn_ps,
                            lhsT=gT_sb[:, mt, :],
                            rhs=expt[:, j * 512:(j + 1) * 512],
                            start=(mtp == 0 and j == 0),
                            stop=(mtp == MT // 2 - 1 and j == 1),
                            skip_group_check=True,
                        )
                    # S accumulation: bf16 quad subtrees chained into an fp32
                    # running sum over the 16 pairs of this pass
                    node, lvl = expt, 0
                    while lvl in quads and lvl < 2:
                        prev = quads.pop(lvl)
                        nt = tree.tile([P, 1024], bf16, tag=f"tree_l{lvl}")
                        nc.vector.tensor_add(nt, prev, node)
                        node, lvl = nt, lvl + 1
                    if lvl < 2:
                        quads[lvl] = node
                    else:
                        last = mtp == MT // 2 - 1
                        if chain is None:
                            chain = node
                        else:
                            nt = tree.tile(
                                [P, 1024], f32, tag="sroot" if last else "chain"
                            )
                            nc.vector.tensor_add(nt, chain, node)
                            chain = nt
                    # drip-feed the previous pass's epilogue between iterations
                    if pending is not None and mtp % 3 == 2:
                        next(pending, None)
                assert not quads
                # fold pair halves: S_bf = root[0:512] + root[512:1024]
                S_bf = tree.tile([P, NQ], bf16, tag="sfold")
                nc.vector.tensor_add(S_bf, chain[:, 0:512], chain[:, 512:1024])
                if pending is not None:
                    for _ in pending:
                        pass
                pending = epilogue_pieces(nh, attn_ps, S_bf)
            for _ in pending:
                pass

    nc.compile()
    return nc


def get_nc():
    if "nc" not in _CACHE:
        _CACHE["nc"] = build_bass()
    return _CACHE["nc"]


def make_in_maps(x, Wt, Wp, Wg, Wo, gamma):
    import ml_dtypes

    bf16 = ml_dtypes.bfloat16
    x = np.asarray(x, dtype=np.float32)
    wt = np.ascontiguousarray(np.asarray(Wt, np.float32).T).astype(bf16)
    wp = np.ascontiguousarray(np.asarray(Wp, np.float32).T).astype(bf16)
    wg = np.ascontiguousarray(np.asarray(Wg, np.float32).T).astype(bf16)
    wo = np.ascontiguousarray(
        (float(np.asarray(gamma)) * np.asarray(Wo, np.float32)).T
    ).astype(bf16)
    in_maps = []
    for i in range(NCORES):
        b, h = divmod(i, 2)
        xb = x[b]
        if h:
            xb = np.concatenate([xb[:, NH:], xb[:, :NH]], axis=1)
        in_maps.append(
            {
                "x": np.ascontiguousarray(xb).astype(bf16),
                "xq": np.ascontiguousarray(x[b][:, h * NH:(h + 1) * NH]),
                "wt": wt,
                "wp": wp,
                "wg": wg,
                "wo": wo,
            }
        )
    return in_maps


def gather_out(results):
    out = np.empty((B, CH, N), np.float32)
    for i in range(NCORES):
        b, h = divmod(i, 2)
        out[b][:, h * NH:(h + 1) * NH] = results[i]["out"]
    return out


def kernel(x, Wt, Wp, Wg, Wo, gamma):
    _ensure_import_path()
    from concourse.bass_utils import run_bass_kernel_spmd

    nc = get_nc()
    in_maps = make_in_maps(x, Wt, Wp, Wg, Wo, gamma)
    res = run_bass_kernel_spmd(nc, in_maps, core_ids=list(range(NCORES)))
    return gather_out(res.results)


# revision 31
# speedup vs baseline: 1.2858x; 1.2858x over previous
"""Trainium2 Bass kernel for a single-head linear-projection attention block.

Reference computation (B=4, CH=256, N=4096):
    theta = Wt @ x        [B, 32, N]
    phi   = Wp @ x        [B, 32, N]
    g     = Wg @ x        [B, 128, N]
    scores = theta^T phi  [B, N, N]
    beta = softmax(scores, axis=-1)
    attn = g @ beta^T     [B, 128, N]
    out = gamma * (Wo @ attn) + x

Sharding: 8 cores = 4 batches x 2 query-halves. Each core owns one batch's
full sequence (for keys/values) and half the queries. The per-core x is
rotated so its query half is always columns 0:2048, keeping the SPMD program
identical across cores (softmax/attention are invariant to a consistent
permutation of the key axis). No collectives are needed.

Per-core dataflow (all matmuls bf16 with fp32 PSUM accumulation):
  - theta/phi via weight-stationary matmuls, column-packed so both run
    concurrently in distinct PE column groups; each is then replicated to a
    second 32-partition group (SBUF->SBUF DMA) for row-group score packing.
  - gT[m, c] via x-stationary matmuls (g transposed, m on partitions), since
    the attention matmul contracts over m which must sit on partitions; this
    dense back-to-back matmul burst also warms the PE clock (HAM) right
    before the main loop.
  - scoresT[m, n] = phi^T theta computed transposed so that softmax's exp
    output directly feeds the attention matmul without a transpose. Each
    PSUM slot holds a PAIR of m-tiles (one per bank); the two K=32 score
    matmuls run concurrently in distinct PE row groups.
  - exp on the Scalar engine, PSUM -> SBUF bf16 in [128, 1024] instructions
    (no max subtraction needed: |scores| <~ 30). The Scalar engine is the
    critical path; the main loop runs at its throughput (~1.0us/tile).
  - 4 passes of 512 queries; attention accumulated over m in PSUM; softmax
    denominator summed on the Vector engine (bf16 quad subtrees + fp32
    chain), partition-reduced+broadcast with one ones-matmul; normalization
    applied after the Wo projection so the reciprocal overlaps it. Each
    pass's epilogue is dripped into the next pass's loop to keep the PE
    queue gap-free (a >~0.7us PE gap permanently halves the PE clock).
  - gamma folded into Wo on the host; fp32 residual add with x.

Measured on 8 axon-tunneled TRN2 cores: ~104-107us, rel_err 5.5e-3.
"""

import os
import sys

import numpy as np

B, CH, N = 4, 256, 4096
NCORES = 8
NH = N // 2  # queries per core
P = 128

_REPO_CANDIDATES = ["/opt/trn_rl_repo", "/root/.axon_site/_ro/trn_rl_repo"]


def _ensure_import_path():
    try:
        import concourse.bass  # noqa: F401
        return
    except ImportError:
        pass
    for cand in _REPO_CANDIDATES:
        if os.path.isdir(cand):
            sys.path.insert(0, cand)
            try:
                import concourse.bass  # noqa: F401
                return
            except ImportError:
                sys.path.pop(0)
    raise ImportError("could not locate concourse (bass) repo")


_CACHE = {}


def build_bass():
    """Build + compile the per-core Tile program (identical on all 8 cores)."""
    _ensure_import_path()
    import concourse.bacc as bacc
    import concourse.tile as tile
    from concourse import mybir

    dt = mybir.dt
    f32 = dt.float32
    bf16 = dt.bfloat16
    Exp = mybir.ActivationFunctionType.Exp

    nc = bacc.Bacc(
        "TRN2",
        target_bir_lowering=False,
        debug=False,
        num_devices=NCORES,
    )

    # Per-core DRAM I/O.
    x_d = nc.dram_tensor("x", [CH, N], bf16, kind="ExternalInput")
    xq_d = nc.dram_tensor("xq", [CH, NH], f32, kind="ExternalInput")
    wt_d = nc.dram_tensor("wt", [CH, 32], bf16, kind="ExternalInput")   # Wt^T
    wp_d = nc.dram_tensor("wp", [CH, 32], bf16, kind="ExternalInput")   # Wp^T
    wg_d = nc.dram_tensor("wg", [CH, 128], bf16, kind="ExternalInput")  # Wg^T
    wo_d = nc.dram_tensor("wo", [128, CH], bf16, kind="ExternalInput")  # (gamma*Wo)^T
    out_d = nc.dram_tensor("out", [CH, NH], f32, kind="ExternalOutput")

    MT = N // P  # 32 m-tiles
    NQ = 512     # query chunk per pass (4 passes over n)

    with tile.TileContext(nc) as tc:
        with (
            tc.tile_pool(name="const", bufs=1) as const,
            tc.tile_pool(name="xp", bufs=1) as xp,
            tc.tile_pool(name="proj", bufs=1) as proj,
            tc.tile_pool(name="expp", bufs=6) as expp,
            tc.tile_pool(name="acc", bufs=2) as acc,
            tc.tile_pool(name="outp", bufs=1) as outp,
            tc.tile_pool(name="tree", bufs=2) as tree,
            tc.tile_pool(name="ps2", bufs=3, space="PSUM") as ps2,
            tc.tile_pool(name="psA", bufs=2, space="PSUM") as psA,
        ):
            # ---- weights (tiny, scalar-engine HWDGE queue, needed first) ----
            wt_sb = const.tile([P, 2, 32], bf16)
            wp_sb = const.tile([P, 2, 32], bf16)
            wg_sb = const.tile([P, 2, 128], bf16)
            wo_sb = const.tile([P, CH], bf16)
            ones_sb = const.tile([P, P], bf16)
            nc.scalar.dma_start(
                out=wt_sb, in_=wt_d.ap().rearrange("(kb p) m -> p kb m", p=P)
            )
            nc.scalar.dma_start(
                out=wp_sb, in_=wp_d.ap().rearrange("(kb p) m -> p kb m", p=P)
            )
            nc.scalar.dma_start(
                out=wg_sb, in_=wg_d.ap().rearrange("(kb p) m -> p kb m", p=P)
            )
            nc.scalar.dma_start(out=wo_sb, in_=wo_d.ap())
            nc.vector.memset(ones_sb, 1.0)

            # dense dummy matmul burst during the initial x-DMA wait: trips
            # the PE clock monitor (HAM) to full rate before the projections
            warm_sb = const.tile([P, 512], bf16)
            nc.vector.memset(warm_sb, 0.0)
            for _ in range(12):
                ps_w = ps2.tile([P, 512], f32, tag="ps")
                nc.tensor.matmul(ps_w, lhsT=ones_sb, rhs=warm_sb, start=True, stop=True)

            # ---- x in column blocks of 1024 so compute starts early ----
            x_sb = xp.tile([P, 2, N], bf16)
            xq_sb = xp.tile([P, 2, NH], f32)
            for cb in range(4):
                eng = nc.sync if cb < 2 else nc.scalar
                for kb in range(2):
                    eng.dma_start(
                        out=x_sb[:, kb, cb * 1024:(cb + 1) * 1024],
                        in_=x_d[kb * P:(kb + 1) * P, cb * 1024:(cb + 1) * 1024],
                    )

            # ---- projections, theta/phi column-packed (concurrent in PE col
            # groups): theta natively at partitions 0:32, phi at 32:64; the
            # replica DMAs fill the opposite group so both live at 0:64 for
            # the 2x row-group packing of the score matmuls. ----
            theta_sb = proj.tile([64, NH], bf16)
            phi_sb = proj.tile([64, N], bf16)
            gT_sb = proj.tile([P, MT, P], bf16)
            for cb in range(4):
                cbs = slice(cb * 1024, (cb + 1) * 1024)
                ps_p = ps2.tile([64, 1024], f32, tag="ps")
                for c in range(2):
                    for kb in range(2):
                        if cb < 2:
                            nc.tensor.matmul(
                                ps_p[0:32, c * 512:(c + 1) * 512],
                                lhsT=wt_sb[:, kb, :],
                                rhs=x_sb[:, kb, cb * 1024 + c * 512:cb * 1024 + (c + 1) * 512],
                                start=(kb == 0),
                                stop=(kb == 1),
                                skip_group_check=True,
                            )
                        nc.tensor.matmul(
                            ps_p[32:64, c * 512:(c + 1) * 512],
                            lhsT=wp_sb[:, kb, :],
                            rhs=x_sb[:, kb, cb * 1024 + c * 512:cb * 1024 + (c + 1) * 512],
                            start=(kb == 0),
                            stop=(kb == 1),
                            skip_group_check=True,
                        )
                if cb < 2:
                    nc.vector.tensor_copy(out=theta_sb[0:32, cbs], in_=ps_p[0:32, :])
                    nc.scalar.dma_start(
                        out=theta_sb[32:64, cbs], in_=theta_sb[0:32, cbs]
                    )
                nc.vector.tensor_copy(out=phi_sb[32:64, cbs], in_=ps_p[32:64, :])
                nc.scalar.dma_start(out=phi_sb[0:32, cbs], in_=phi_sb[32:64, cbs])

            # ---- gT last: a dense back-to-back matmul burst that warms the
            # PE clock (HAM) right before the gap-free main loop ----
            def emit_gt_group(grp):
                ps_g = ps2.tile([P, 4, P], f32, tag="ps")
                for j in range(4):
                    mt = grp * 4 + j
                    for kb in range(2):
                        nc.tensor.matmul(
                            ps_g[:, j, :],
                            lhsT=x_sb[:, kb, mt * P:(mt + 1) * P],
                            rhs=wg_sb[:, kb, :],
                            start=(kb == 0),
                            stop=(kb == 1),
                        )
                nc.vector.tensor_copy(
                    out=gT_sb[:, grp * 4:(grp + 1) * 4, :], in_=ps_g
                )

            for grp in range(MT // 4 - 1):
                emit_gt_group(grp)

            # residual x slice: only needed by the epilogue, so load it late
            for kb in range(2):
                nc.scalar.dma_start(
                    out=xq_sb[:, kb, :], in_=xq_d[kb * P:(kb + 1) * P, :]
                )

            out_sb = outp.tile([P, 2, NH], f32)

            def epilogue_pieces(nh, attn_ps, S_bf):
                """Piece 0 (the cast) frees the attention PSUM slot for the
                next pass immediately; the rest is dripped into the next
                pass's loop. Epilogue PSUM lives in its own 1-bank pool."""
                A_bf = acc.tile([P, NQ], bf16, tag="abf")
                nc.vector.tensor_copy(out=A_bf, in_=attn_ps)
                yield
                ps_S = psA.tile([P, NQ], f32, tag="attn")
                nc.tensor.matmul(ps_S, lhsT=ones_sb, rhs=S_bf, start=True, stop=True)
                yield
                recip = acc.tile([P, NQ], f32, tag="recip")
                nc.vector.reciprocal_approx_fast(out=recip, in_=ps_S)
                yield
                sl = slice(nh * NQ, (nh + 1) * NQ)
                for oc in range(2):
                    ps_o = psA.tile([P, NQ], f32, tag="attn")
                    nc.tensor.matmul(
                        ps_o,
                        lhsT=wo_sb[:, oc * P:(oc + 1) * P],
                        rhs=A_bf,
                        start=True,
                        stop=True,
                    )
                    tmp = acc.tile([P, NQ], f32, tag="tmp")
                    nc.vector.tensor_mul(tmp, ps_o, recip)
                    nc.vector.tensor_add(out_sb[:, oc, sl], tmp, xq_sb[:, oc, sl])
                    nc.sync.dma_start(
                        out=out_d[oc * P:(oc + 1) * P, sl], in_=out_sb[:, oc, sl]
                    )
                    yield

            # ---- four passes over the query axis (512 queries each). Each
            # PSUM slot holds one PAIR of m-tiles (mtE in cols 0:512 = bank 0,
            # mtO in cols 512:1024 = bank 1): the two row-group score matmuls
            # become ready together and run concurrently in the PE array.
            # Each pass's epilogue interleaves into the next pass's loop. ----
            pending = None
            for nh in range(4):
                attn_ps = psA.tile([P, NQ], f32, tag="attn")
                quads = {}
                chain = None
                for mtp in range(MT // 2):
                    mtE, mtO = mtp * 2, mtp * 2 + 1
                    ns = slice(nh * NQ, (nh + 1) * NQ)
                    ps_s = ps2.tile([P, 1024], f32, tag="ps")
                    for j, mt in ((0, mtE), (1, mtO)):
                        nc.tensor.matmul(
                            ps_s[:, j * 512:(j + 1) * 512],
                            lhsT=phi_sb[32 * j:32 * (j + 1), mt * P:(mt + 1) * P],
                            rhs=theta_sb[32 * j:32 * (j + 1), ns],
                            start=True,
                            stop=True,
                        )
                    if nh == 0 and mtp == 0:
                        # dense PE bridge over the first exp's latency
                        emit_gt_group(MT // 4 - 1)
                    expt = expp.tile([P, 1024], bf16, tag="expt")
                    nc.scalar.activation(out=expt, in_=ps_s, func=Exp)
                    for j, mt in ((0, mtE), (1, mtO)):
                        nc.tensor.matmul(
                            attn_ps,
                            lhsT=gT_sb[:, mt, :],
                            rhs=expt[:, j * 512:(j + 1) * 512],
                            start=(mtp == 0 and j == 0),
                            stop=(mtp == MT // 2 - 1 and j == 1),
                            skip_group_check=True,
                        )
                    # S accumulation: bf16 quad subtrees chained into an fp32
                    # running sum over the 16 pairs of this pass
                    node, lvl = expt, 0
                    while lvl in quads and lvl < 2:
                        prev = quads.pop(lvl)
                        nt = tree.tile([P, 1024], bf16, tag=f"tree_l{lvl}")
                        nc.vector.tensor_add(nt, prev, node)
                        node, lvl = nt, lvl + 1
                    if lvl < 2:
                        quads[lvl] = node
                    else:
                        last = mtp == MT // 2 - 1
                        if chain is None:
                            chain = node
                        else:
                            nt = tree.tile(
                                [P, 1024], f32, tag="sroot" if last else "chain"
                            )
                            nc.vector.tensor_add(nt, chain, node)
                            chain = nt
                    # drip-feed the previous pass's epilogue between iterations
                    if pending is not None and mtp % 3 == 2:
                        next(pending, None)
                assert not quads
                # fold pair halves: S_bf = root[0:512] + root[512:1024]
                S_bf = tree.tile([P, NQ], bf16, tag="sfold")
                nc.vector.tensor_add(S_bf, chain[:, 0:512], chain[:, 512:1024])
                if pending is not None:
                    for _ in pending:
                        pass
                pending = epilogue_pieces(nh, attn_ps, S_bf)
            for _ in pending:
                pass

    nc.compile()
    return nc


def get_nc():
    if "nc" not in _CACHE:
        _CACHE["nc"] = build_bass()
    return _CACHE["nc"]


def make_in_maps(x, Wt, Wp, Wg, Wo, gamma):
    import ml_dtypes

    bf16 = ml_dtypes.bfloat16
    x = np.asarray(x, dtype=np.float32)
    wt = np.ascontiguousarray(np.asarray(Wt, np.float32).T).astype(bf16)
    wp = np.ascontiguousarray(np.asarray(Wp, np.float32).T).astype(bf16)
    wg = np.ascontiguousarray(np.asarray(Wg, np.float32).T).astype(bf16)
    wo = np.ascontiguousarray(
        (float(np.asarray(gamma)) * np.asarray(Wo, np.float32)).T
    ).astype(bf16)
    in_maps = []
    for i in range(NCORES):
        b, h = divmod(i, 2)
        xb = x[b]
        if h:
            xb = np.concatenate([xb[:, NH:], xb[:, :NH]], axis=1)
        in_maps.append(
            {
                "x": np.ascontiguousarray(xb).astype(bf16),
                "xq": np.ascontiguousarray(x[b][:, h * NH:(h + 1) * NH]),
                "wt": wt,
                "wp": wp,
                "wg": wg,
                "wo": wo,
            }
        )
    return in_maps


def gather_out(results):
    out = np.empty((B, CH, N), np.float32)
    for i in range(NCORES):
        b, h = divmod(i, 2)
        out[b][:, h * NH:(h + 1) * NH] = results[i]["out"]
    return out


def kernel(x, Wt, Wp, Wg, Wo, gamma):
    _ensure_import_path()
    from concourse.bass_utils import run_bass_kernel_spmd

    nc = get_nc()
    in_maps = make_in_maps(x, Wt, Wp, Wg, Wo, gamma)
    res = run_bass_kernel_spmd(nc, in_maps, core_ids=list(range(NCORES)))
    return gather_out(res.results)
